# revision 24
# baseline (speedup 1.0000x reference)
"""Trainium2 Bass kernel for nn_ActionHead (ragged_sequence).

Reference computation (per batch b of 64, U=512 unit slots):
    combined = [own_unit_embeddings[b] | broadcast(core_output[b])]   # [512, 768]
    h        = relu(combined @ W1 + b1)                               # [512, 1024]  (output 2)
    logits   = h @ W2 + b2                                            # [512, 12]
    probs    = softmax(logits) * mask(nr_own_flags[b] <= t < nr_units[b], 1.0 / 1e-9)

Sharding: data-parallel over batch, 8 batches per core on 8 cores. Weights
replicated.

Device-side layout strategy: everything is computed transposed (tokens on the
matmul free dim), which makes the FC1 bias a per-partition ACT bias and avoids
all on-device transposes; the host transposes the outputs back (pure data
marshalling, not compute). The broadcast core_output contribution to FC1
collapses into a per-batch [1024] vector (core @ W1[256:]) computed once, so
the dense per-token FC1 contraction is only K=256.

Per core (software-pipelined FC2(i-2) | sum(i-3) | fin(i-4) | FC1(i) so the
PE stream stays dense and HAM-warm):
  corevecT[b, j] = core_b @ W1c (wide-N matmuls + small PE transposes);
                   bias[j, b] = corevecT.T + b1[j]
  per batch b:  hT[j, t]  = relu(sum_f embT[f, t] W1e[f, j] + bias[j, b])
                            (PE; evacuation split ACT relu-bias / DVE 2-op
                            tensor_scalar), streamed to DRAM as computed
                logitsT   = sum_j W2pad[j, a] hT[j, t]   [64, U] at base 0
                            (f32r matmuls only compile with PSUM dst
                            partition base 0 - fp32 pair-pass col groups)
                softmax over the 12 logit rows (partition axis) without
                transposes: exp (ACT, +b2 bias), denominator via ones-vector
                matmul (PE), reciprocal_approx_fast (DVE), ragged mask fold
                (DVE), partition-broadcast via K=1 matmul (PE), final
                product (DVE).

Matmul operands are float32r (streams at 1 cycle/row vs 4 for float32); the
BIR verifier requires f32r-consumed data to be produced as f32r, so those
tiles/DRAM params are declared float32r end-to-end (same bytes as float32 on
the host side).
"""

import os
import sys

for _p in ("/opt/trn_rl_repo", "/root/.axon_site/_ro/trn_rl_repo"):
    if os.path.isdir(_p) and _p not in sys.path:
        sys.path.append(_p)

import numpy as np
from contextlib import ExitStack

import concourse.bacc as bacc
import concourse.bass as bass
import concourse.mybir as mybir
import concourse.tile as tile
from concourse.bass_utils import run_bass_kernel_spmd

F32 = mybir.dt.float32
F32R = mybir.dt.float32r
AF = mybir.ActivationFunctionType
ALU = mybir.AluOpType
ts = bass.ts

NCORES = 8
B = 64
BP = B // NCORES      # batches per core = 8
U = 512               # unit slots (tokens) per batch
UNIT = 256            # unit embedding size
CORE = 512            # core output size
HID = 1024
ACTN = 12
KE = UNIT // 128      # 2  k-tiles for the emb part of FC1
KC = CORE // 128      # 4  k-tiles for the core part of FC1
JT = HID // 128       # 8  j-tiles
# f32r matmuls only compile with PSUM dst partition base 0 (the internal
# fp32 pair-pass doubles the column groups), so no batch stacking: softmax
# runs per batch on [64, U] logits tiles at base 0.

# matmul operand dtype: float32r streams at 1 cycle/row (vs 4 for float32).
DT = F32R
# dtype of the hT activations (output h AND FC2 input). If f32r rounding is
# too lossy for the h output, set to F32 and FC2 runs on a separate cast.
H_DT = F32R


def _build():
    nc = bacc.Bacc("TRN2", target_bir_lowering=False, debug=False)

    # ---- DRAM I/O (per core) ----
    embT = nc.dram_tensor("embT", [BP, UNIT, U], DT, kind="ExternalInput").ap()
    coreT = nc.dram_tensor("coreT", [CORE, BP], DT, kind="ExternalInput").ap()
    w1e = nc.dram_tensor("w1e", [UNIT, HID], DT, kind="ExternalInput").ap()
    w1c = nc.dram_tensor("w1c", [CORE, HID], DT, kind="ExternalInput").ap()
    w2p = nc.dram_tensor("w2p", [128, JT * 32], DT, kind="ExternalInput").ap()
    b1r = nc.dram_tensor("b1r", [128, JT], F32, kind="ExternalInput").ap()
    b2s = nc.dram_tensor("b2s", [128, 1], F32, kind="ExternalInput").ap()
    sum12 = nc.dram_tensor("sum12", [32, 1], DT, kind="ExternalInput").ap()
    id8 = nc.dram_tensor("id8", [8, 8], DT, kind="ExternalInput").ap()
    ones12 = nc.dram_tensor("ones12", [1, ACTN], DT, kind="ExternalInput").ap()
    maskf = nc.dram_tensor("maskf", [1, BP * U], F32, kind="ExternalInput").ap()

    hT_out = nc.dram_tensor("hT_out", [BP, HID, U], H_DT, kind="ExternalOutput").ap()
    probsT_out = nc.dram_tensor(
        "probsT_out", [BP, ACTN, U], F32, kind="ExternalOutput"
    ).ap()

    with tile.TileContext(nc) as tc, ExitStack() as ctx:
        consts = ctx.enter_context(tc.tile_pool(name="consts", bufs=1))

        # ---- constant loads (queue order = criticality: corevec inputs
        # first in per-jt chunks, then FC1's w1e; w2p only matters at FC2(0))
        # one sync load queue, criticality order: the bias chain (coreT,
        # w1c) gates every relu evacuation, so it loads first; the corevec
        # matmuls then overlap the w1e/embT(0) loads that FC1(0) waits on.
        # tiny consts go on the scalar HWDGE queue (parallel issue, ~0 BW);
        # the sync queue carries the big critical-path loads in order, with
        # w1c in halves so the corevec/bias chain starts after 1 MB.
        coreT_t = consts.tile([128, KC, BP], DT, name="coreT_t", tag="coreT")
        nc.scalar.dma_start(out=coreT_t, in_=coreT.rearrange("(k p) b -> p k b", p=128))
        b1r_t = consts.tile([128, JT], F32, name="b1r_t", tag="b1r")
        nc.scalar.dma_start(out=b1r_t, in_=b1r)
        id8_t = consts.tile([8, 8], DT, name="id8_t", tag="id8")
        nc.scalar.dma_start(out=id8_t, in_=id8)
        w2p_t = consts.tile([128, JT * 32], DT, name="w2p_t", tag="w2p")
        nc.scalar.dma_start(out=w2p_t, in_=w2p)
        b2s_t = consts.tile([128, 1], F32, name="b2s_t", tag="b2s")
        nc.scalar.dma_start(out=b2s_t, in_=b2s)
        sum12_t = consts.tile([32, 1], DT, name="sum12_t", tag="sum12")
        nc.scalar.dma_start(out=sum12_t, in_=sum12)
        ones12_t = consts.tile([1, ACTN], DT, name="ones12_t", tag="ones12")
        nc.scalar.dma_start(out=ones12_t, in_=ones12)
        # ragged mask rows, one per batch along the free dim (engine operands
        # need 32-aligned partition bases, so [8, U] with row slicing is out)
        mfin_t = consts.tile([1, BP * U], F32, name="mfin_t", tag="mfin")
        nc.scalar.dma_start(out=mfin_t, in_=maskf)

        w1c_t = consts.tile([128, KC, HID], DT, name="w1c_t", tag="w1c")
        nc.sync.dma_start(
            out=w1c_t[:, :, 0:512],
            in_=w1c[:, 0:512].rearrange("(k p) j -> p k j", p=128),
        )
        w1e_t = consts.tile([128, KE, HID], DT, name="w1e_t", tag="w1e")
        nc.sync.dma_start(out=w1e_t, in_=w1e.rearrange("(k p) j -> p k j", p=128))
        embp = ctx.enter_context(tc.tile_pool(name="embp", bufs=4))
        e_first = embp.tile([128, KE, U], DT, name="e_0", tag="e")
        nc.sync.dma_start(out=e_first, in_=embT[0].rearrange("(k p) t -> p k t", p=128))
        nc.sync.dma_start(
            out=w1c_t[:, :, 512:1024],
            in_=w1c[:, 512:1024].rearrange("(k p) j -> p k j", p=128),
        )

        # FC1 bias tile, filled by emit_corevec (scheduled after FC1(0) so the
        # first batch's matmuls start as soon as w1e+embT arrive, without
        # waiting on the 2 MB w1c load that only the bias needs)
        bias_t = consts.tile([128, JT * BP], F32, name="bias_t", tag="bias")

        # corevecT[b, j] = core_b @ W1c via wide-N matmuls (LDW is only 8
        # columns), then 8 small PE transposes to the [j, b] layout the
        # per-partition bias slot needs. Runs while w1e/embT(0) still load.
        with tc.tile_pool(name="cvps", bufs=1, space="PSUM") as cvps:
            cvT_sb = consts.tile([8, HID], DT, name="cvT_sb", tag="cvT_sb")
            for nh in range(2):
                ps_cvT = cvps.tile([8, 512], F32, name=f"ps_cvT{nh}", tag="cvT")
                for kc in range(KC):
                    nc.tensor.matmul(
                        ps_cvT,
                        lhsT=coreT_t[:, kc, :],
                        rhs=w1c_t[:, kc, ts(nh, 512)],
                        start=(kc == 0),
                        stop=(kc == KC - 1),
                    )
                nc.vector.tensor_copy(cvT_sb[:, ts(nh, 512)], ps_cvT)
                for jt in range(nh * 4, nh * 4 + 4):
                    tp = cvps.tile([128, BP], DT, name=f"tp{jt}", tag="tp", bufs=2)
                    nc.tensor.transpose(tp, cvT_sb[:, ts(jt, 128)], id8_t)
                    nc.vector.tensor_scalar_add(
                        bias_t[:, jt * BP : (jt + 1) * BP], tp,
                        b1r_t[:, jt : jt + 1],
                    )

        # ---- pools ----
        hp = ctx.enter_context(tc.tile_pool(name="hp", bufs=3))
        fc1ps = ctx.enter_context(tc.tile_pool(name="fc1ps", bufs=4, space="PSUM"))

        hT = [None] * BP
        epairs = [None] * BP
        psL = [None] * BP
        expS = [None] * BP
        scl = [None] * BP

        NDVE = 4  # j-tiles whose relu evacuation runs on DVE instead of ACT

        def stage_fc1(b):
            """load embT, FC1 matmuls, relu evacuation, hT writeback."""
            if b == 0:
                e = e_first
            elif b % 2 == 1:
                nb2 = min(2, BP - b)
                epair = embp.tile([128, nb2, KE, U], DT, name=f"e_{b}", tag="ep")
                nc.sync.dma_start(
                    out=epair,
                    in_=embT[b : b + nb2].rearrange("b (k p) t -> p b k t", p=128),
                )
                epairs[b] = epair
                e = epair[:, 0]
            else:
                e = epairs[b - 1][:, 1]
            hT[b] = hp.tile([128, JT, U], H_DT, name=f"hT_{b}", tag="hT")
            # the last batch's stores go on the (idle by then) sync queue, in
            # quarters so the final store latency is short
            if b == BP - 1 or (b >= 4 and b % 2 == 0):
                store_eng = nc.sync
            else:
                store_eng = nc.gpsimd
            nstores = 4 if b == BP - 1 else 2
            jper = JT // nstores
            for jt in range(JT):
                ps = fc1ps.tile([128, U], F32, name=f"ps_{b}_{jt}", tag="fc1")
                nc.tensor.matmul(
                    ps, lhsT=w1e_t[:, 0, ts(jt, 128)], rhs=e[:, 0, :],
                    start=True, stop=False,
                )
                nc.tensor.matmul(
                    ps, lhsT=w1e_t[:, 1, ts(jt, 128)], rhs=e[:, 1, :],
                    start=False, stop=True,
                )
                bias_ap = bias_t[:, jt * BP + b : jt * BP + b + 1]
                if jt < JT - NDVE:
                    nc.scalar.activation(hT[b][:, jt, :], ps, AF.Relu, bias=bias_ap)
                else:
                    nc.vector.tensor_scalar(
                        out=hT[b][:, jt, :], in0=ps, scalar1=bias_ap,
                        scalar2=0.0, op0=ALU.add, op1=ALU.max,
                    )
                if (jt + 1) % jper == 0:
                    q = jt // jper
                    store_eng.dma_start(
                        out=hT_out[b, ts(q, jper * 128), :].rearrange(
                            "(k p) t -> p k t", p=128
                        ),
                        in_=hT[b][:, ts(q, jper), :],
                    )

        # remaining PSUM pools (reuse the corevec banks)
        lps = ctx.enter_context(tc.tile_pool(name="lps", bufs=2, space="PSUM"))
        smps = ctx.enter_context(tc.tile_pool(name="smps", bufs=1, space="PSUM"))
        bps = ctx.enter_context(tc.tile_pool(name="bps", bufs=1, space="PSUM"))
        smx = ctx.enter_context(tc.tile_pool(name="smx", bufs=2))

        def stage_fc2(b):
            """FC2 matmuls into [64, U] logits (rows 12..63 zero pad) + exp."""
            psL[b] = lps.tile([32, U], F32, name=f"psL_{b}", tag="psL")
            for jt in range(JT):
                nc.tensor.matmul(
                    psL[b], lhsT=w2p_t[:, ts(jt, 32)], rhs=hT[b][:, jt, :],
                    start=(jt == 0), stop=(jt == JT - 1),
                )
            expS[b] = smx.tile([32, U], DT, name=f"exp_{b}", tag="exp", bufs=4)
            nc.scalar.activation(expS[b], psL[b], AF.Exp, bias=b2s_t[0:32, 0:1])

        def stage_sum(b):
            """per-t softmax denominator, reciprocal, ragged-mask fold."""
            ps_sum = smps.tile([1, U], F32, name=f"pssum_{b}", tag="pssum")
            nc.tensor.matmul(ps_sum, lhsT=sum12_t, rhs=expS[b], start=True, stop=True)
            recip = smx.tile([1, U], F32, name=f"recip_{b}", tag="recip")
            nc.vector.reciprocal_approx_fast(out=recip, in_=ps_sum)
            scl[b] = smx.tile([1, U], DT, name=f"scaled_{b}", tag="scaled")
            nc.vector.tensor_mul(scl[b], recip, mfin_t[:, b * U : (b + 1) * U])

        def stage_fin(b):
            """broadcast scale across the 12 logit rows, final probs, out."""
            ps_b = bps.tile([ACTN, U], F32, name=f"psb_{b}", tag="psb")
            nc.tensor.matmul(ps_b, lhsT=ones12_t, rhs=scl[b], start=True, stop=True)
            probsT = smx.tile([ACTN, U], F32, name=f"probsT_{b}", tag="probsT")
            nc.vector.tensor_mul(probsT, expS[b][0:ACTN, :].bitcast(F32), ps_b)
            (nc.sync if b >= BP - 2 else nc.gpsimd).dma_start(
                out=probsT_out[b], in_=probsT
            )

        # steady pipeline: FC2(i-2) | sum(i-3) | fin(i-4) | FC1(i)
        stage_fc1(0)
        stage_fc1(1)
        for i in range(2, BP + 4):
            if 0 <= i - 2 < BP:
                stage_fc2(i - 2)
            if 0 <= i - 3 < BP:
                stage_sum(i - 3)
            if 0 <= i - 4 < BP:
                stage_fin(i - 4)
            if i < BP:
                stage_fc1(i)

    nc.compile()
    return nc


_NC = None


def _get_nc():
    global _NC
    if _NC is None:
        _NC = _build()
    return _NC


def _make_in_maps(core_output, own_unit_embeddings, nr_units, nr_own_flags,
                  W1, b1, W2, b2):
    f32 = np.float32
    emb = np.ascontiguousarray(own_unit_embeddings, dtype=f32)
    core = np.ascontiguousarray(core_output, dtype=f32).reshape(B, CORE)
    W1 = np.ascontiguousarray(W1, dtype=f32)
    b1 = np.ascontiguousarray(b1, dtype=f32)
    W2 = np.ascontiguousarray(W2, dtype=f32)
    b2 = np.ascontiguousarray(b2, dtype=f32)
    units = np.asarray(nr_units).reshape(B).astype(f32)
    flags = np.asarray(nr_own_flags).reshape(B).astype(f32)

    w1e = np.ascontiguousarray(W1[:UNIT])
    w1c = np.ascontiguousarray(W1[UNIT:])
    w2p = np.zeros((128, JT * 32), f32)
    for k in range(JT):
        w2p[:, 32 * k : 32 * k + ACTN] = W2[128 * k : 128 * (k + 1)]
    b1r = np.ascontiguousarray(b1.reshape(JT, 128).T)
    b2s = np.zeros((128, 1), f32)
    b2s[:ACTN, 0] = b2
    sum12 = np.zeros((32, 1), f32)
    sum12[:ACTN, 0] = 1.0
    ones12 = np.ones((1, ACTN), f32)
    id8 = np.eye(8, dtype=f32)
    idx = np.arange(U, dtype=f32)[None, :]
    valid = (idx >= flags[:, None]) & (idx < units[:, None])       # [B, U]
    maskv = np.where(valid, np.float32(1.0), np.float32(1e-9))

    shared = dict(w1e=w1e, w1c=w1c, w2p=w2p, b1r=b1r, b2s=b2s, sum12=sum12,
                  ones12=ones12, id8=id8)

    in_maps = []
    for c in range(NCORES):
        sl = slice(c * BP, (c + 1) * BP)
        embT_c = np.ascontiguousarray(emb[sl].transpose(0, 2, 1))
        coreT_c = np.ascontiguousarray(core[sl].T)
        maskf_c = np.ascontiguousarray(maskv[sl].reshape(1, BP * U))
        in_maps.append(dict(embT=embT_c, coreT=coreT_c, maskf=maskf_c, **shared))
    return in_maps


def _gather(results):
    probs = np.empty((B, U, ACTN), np.float32)
    h = np.empty((B, U, HID), np.float32)
    for c, r in enumerate(results):
        sl = slice(c * BP, (c + 1) * BP)
        h[sl] = r["hT_out"].transpose(0, 2, 1)
        probs[sl] = r["probsT_out"].transpose(0, 2, 1)
    return probs, h


def run(inputs, trace=False, **kw):
    """Build + run on 8 cores. Returns (probs, h), BassKernelResults."""
    nc = _get_nc()
    in_maps = _make_in_maps(**inputs)
    res = run_bass_kernel_spmd(nc, in_maps, list(range(NCORES)), trace=trace, **kw)
    return _gather(res.results), res


def kernel(core_output, own_unit_embeddings, nr_units, nr_own_flags, W1, b1, W2, b2):
    (probs, h), _ = run(dict(
        core_output=core_output, own_unit_embeddings=own_unit_embeddings,
        nr_units=nr_units, nr_own_flags=nr_own_flags, W1=W1, b1=b1, W2=W2, b2=b2,
    ))
    return probs, h


# revision 26
# speedup vs baseline: 1.1836x; 1.1836x over previous
"""Trainium2 Bass kernel for nn_ActionHead (ragged_sequence).

Reference computation (per batch b of 64, U=512 unit slots):
    combined = [own_unit_embeddings[b] | broadcast(core_output[b])]   # [512, 768]
    h        = relu(combined @ W1 + b1)                               # [512, 1024]  (output 2)
    logits   = h @ W2 + b2                                            # [512, 12]
    probs    = softmax(logits) * mask(nr_own_flags[b] <= t < nr_units[b], 1.0 / 1e-9)

Sharding: data-parallel over batch, 8 batches per core on 8 cores. Weights
replicated.

Device-side layout strategy: everything is computed transposed (tokens on the
matmul free dim), which makes the FC1 bias a per-partition ACT bias and avoids
all on-device transposes; the host transposes the outputs back (pure data
marshalling, not compute). The broadcast core_output contribution to FC1
collapses into a per-batch [1024] vector (core @ W1[256:]) computed once, so
the dense per-token FC1 contraction is only K=256.

Per core (software-pipelined FC2(i-2) | sum(i-3) | fin(i-4) | FC1(i) so the
PE stream stays dense and HAM-warm):
  corevecT[b, j] = core_b @ W1c (wide-N matmuls + small PE transposes);
                   bias[j, b] = corevecT.T + b1[j]
  per batch b:  hT[j, t]  = relu(sum_f embT[f, t] W1e[f, j] + bias[j, b])
                            (PE; evacuation split ACT relu-bias / DVE 2-op
                            tensor_scalar), streamed to DRAM as computed
                logitsT   = sum_j W2pad[j, a] hT[j, t]   [64, U] at base 0
                            (f32r matmuls only compile with PSUM dst
                            partition base 0 - fp32 pair-pass col groups)
                softmax over the 12 logit rows (partition axis) without
                transposes: exp (ACT, +b2 bias), denominator via ones-vector
                matmul (PE), reciprocal_approx_fast (DVE), ragged mask fold
                (DVE), partition-broadcast via K=1 matmul (PE), final
                product (DVE).

Matmul operands are float32r (streams at 1 cycle/row vs 4 for float32); the
BIR verifier requires f32r-consumed data to be produced as f32r, so those
tiles/DRAM params are declared float32r end-to-end (same bytes as float32 on
the host side).
"""

import os
import sys

for _p in ("/opt/trn_rl_repo", "/root/.axon_site/_ro/trn_rl_repo"):
    if os.path.isdir(_p) and _p not in sys.path:
        sys.path.append(_p)

import numpy as np
from contextlib import ExitStack

import concourse.bacc as bacc
import concourse.bass as bass
import concourse.mybir as mybir
import concourse.tile as tile
from concourse.bass_utils import run_bass_kernel_spmd

F32 = mybir.dt.float32
F32R = mybir.dt.float32r
AF = mybir.ActivationFunctionType
ALU = mybir.AluOpType
ts = bass.ts

NCORES = 8
B = 64
BP = B // NCORES      # batches per core = 8
U = 512               # unit slots (tokens) per batch
UNIT = 256            # unit embedding size
CORE = 512            # core output size
HID = 1024
ACTN = 12
KE = UNIT // 128      # 2  k-tiles for the emb part of FC1
KC = CORE // 128      # 4  k-tiles for the core part of FC1
JT = HID // 128       # 8  j-tiles
# f32r matmuls only compile with PSUM dst partition base 0 (the internal
# fp32 pair-pass doubles the column groups), so no batch stacking: softmax
# runs per batch on [64, U] logits tiles at base 0.

# matmul operand dtype: float32r streams at 1 cycle/row (vs 4 for float32).
DT = F32R
# dtype of the hT activations (output h AND FC2 input). If f32r rounding is
# too lossy for the h output, set to F32 and FC2 runs on a separate cast.
H_DT = F32R


def _build():
    nc = bacc.Bacc("TRN2", target_bir_lowering=False, debug=False)

    # ---- DRAM I/O (per core) ----
    embT = nc.dram_tensor("embT", [BP, UNIT, U], DT, kind="ExternalInput").ap()
    coreT = nc.dram_tensor("coreT", [CORE, BP], DT, kind="ExternalInput").ap()
    w1e = nc.dram_tensor("w1e", [UNIT, HID], DT, kind="ExternalInput").ap()
    w1c = nc.dram_tensor("w1c", [CORE, HID], DT, kind="ExternalInput").ap()
    w2p = nc.dram_tensor("w2p", [128, JT * 32], DT, kind="ExternalInput").ap()
    b1r = nc.dram_tensor("b1r", [128, JT], F32, kind="ExternalInput").ap()
    b2s = nc.dram_tensor("b2s", [128, 1], F32, kind="ExternalInput").ap()
    sum12 = nc.dram_tensor("sum12", [32, 1], DT, kind="ExternalInput").ap()
    id8 = nc.dram_tensor("id8", [8, 8], DT, kind="ExternalInput").ap()
    ones12 = nc.dram_tensor("ones12", [1, ACTN], DT, kind="ExternalInput").ap()
    maskf = nc.dram_tensor("maskf", [1, BP * U], F32, kind="ExternalInput").ap()

    hT_out = nc.dram_tensor("hT_out", [BP, HID, U], H_DT, kind="ExternalOutput").ap()
    probsT_out = nc.dram_tensor(
        "probsT_out", [BP, ACTN, U], F32, kind="ExternalOutput"
    ).ap()

    with tile.TileContext(nc) as tc, ExitStack() as ctx:
        consts = ctx.enter_context(tc.tile_pool(name="consts", bufs=1))

        # ---- constant loads (queue order = criticality: corevec inputs
        # first in per-jt chunks, then FC1's w1e; w2p only matters at FC2(0))
        # one sync load queue, criticality order: the bias chain (coreT,
        # w1c) gates every relu evacuation, so it loads first; the corevec
        # matmuls then overlap the w1e/embT(0) loads that FC1(0) waits on.
        # tiny consts go on the scalar HWDGE queue (parallel issue, ~0 BW);
        # the sync queue carries the big critical-path loads in order, with
        # w1c in halves so the corevec/bias chain starts after 1 MB.
        coreT_t = consts.tile([128, KC, BP], DT, name="coreT_t", tag="coreT")
        nc.scalar.dma_start(out=coreT_t, in_=coreT.rearrange("(k p) b -> p k b", p=128))
        b1r_t = consts.tile([128, JT], F32, name="b1r_t", tag="b1r")
        nc.scalar.dma_start(out=b1r_t, in_=b1r)
        id8_t = consts.tile([8, 8], DT, name="id8_t", tag="id8")
        nc.scalar.dma_start(out=id8_t, in_=id8)
        w2p_t = consts.tile([128, JT * 32], DT, name="w2p_t", tag="w2p")
        nc.scalar.dma_start(out=w2p_t, in_=w2p)
        b2s_t = consts.tile([128, 1], F32, name="b2s_t", tag="b2s")
        nc.scalar.dma_start(out=b2s_t, in_=b2s)
        sum12_t = consts.tile([32, 1], DT, name="sum12_t", tag="sum12")
        nc.scalar.dma_start(out=sum12_t, in_=sum12)
        ones12_t = consts.tile([1, ACTN], DT, name="ones12_t", tag="ones12")
        nc.scalar.dma_start(out=ones12_t, in_=ones12)
        # ragged mask rows, one per batch along the free dim (engine operands
        # need 32-aligned partition bases, so [8, U] with row slicing is out)
        mfin_t = consts.tile([1, BP * U], F32, name="mfin_t", tag="mfin")
        nc.scalar.dma_start(out=mfin_t, in_=maskf)

        w1e_t = consts.tile([128, KE, HID], DT, name="w1e_t", tag="w1e")
        nc.sync.dma_start(out=w1e_t, in_=w1e.rearrange("(k p) j -> p k j", p=128))
        w1c_t = consts.tile([128, KC, HID], DT, name="w1c_t", tag="w1c")
        nc.sync.dma_start(
            out=w1c_t[:, :, 0:512],
            in_=w1c[:, 0:512].rearrange("(k p) j -> p k j", p=128),
        )
        embp = ctx.enter_context(tc.tile_pool(name="embp", bufs=4))
        e_first = embp.tile([128, KE, U], DT, name="e_0", tag="e")
        nc.sync.dma_start(out=e_first, in_=embT[0].rearrange("(k p) t -> p k t", p=128))
        nc.sync.dma_start(
            out=w1c_t[:, :, 512:1024],
            in_=w1c[:, 512:1024].rearrange("(k p) j -> p k j", p=128),
        )

        # FC1 bias tile, filled by emit_corevec (scheduled after FC1(0) so the
        # first batch's matmuls start as soon as w1e+embT arrive, without
        # waiting on the 2 MB w1c load that only the bias needs)
        bias_t = consts.tile([128, JT * BP], F32, name="bias_t", tag="bias")

        # corevecT[b, j] = core_b @ W1c via wide-N matmuls (LDW is only 8
        # columns), then 8 small PE transposes to the [j, b] layout the
        # per-partition bias slot needs. Runs while w1e/embT(0) still load.
        cvT_sb = consts.tile([8, HID], DT, name="cvT_sb", tag="cvT_sb")

        def emit_corevec(nh, cvps):
            """bias[j, b] for j-half nh: core @ W1c half + transpose + b1."""
            ps_cvT = cvps.tile([8, 512], F32, name=f"ps_cvT{nh}", tag="cvT")
            for kc in range(KC):
                nc.tensor.matmul(
                    ps_cvT,
                    lhsT=coreT_t[:, kc, :],
                    rhs=w1c_t[:, kc, ts(nh, 512)],
                    start=(kc == 0),
                    stop=(kc == KC - 1),
                )
            nc.vector.tensor_copy(cvT_sb[:, ts(nh, 512)], ps_cvT)
            for jt in range(nh * 4, nh * 4 + 4):
                tp = cvps.tile([128, BP], DT, name=f"tp{jt}", tag="tp", bufs=2)
                nc.tensor.transpose(tp, cvT_sb[:, ts(jt, 128)], id8_t)
                nc.vector.tensor_scalar_add(
                    bias_t[:, jt * BP : (jt + 1) * BP], tp,
                    b1r_t[:, jt : jt + 1],
                )

        # ---- pools ----
        hp = ctx.enter_context(tc.tile_pool(name="hp", bufs=3))
        fc1ps = ctx.enter_context(tc.tile_pool(name="fc1ps", bufs=4, space="PSUM"))

        hT = [None] * BP
        epairs = [None] * BP
        psL = [None] * BP
        expS = [None] * BP
        scl = [None] * BP

        NDVE = 4  # j-tiles whose relu evacuation runs on DVE instead of ACT

        def stage_fc1(b):
            """load embT, FC1 matmuls, relu evacuation, hT writeback."""
            if b == 0:
                e = e_first
            elif b % 2 == 1:
                nb2 = min(2, BP - b)
                epair = embp.tile([128, nb2, KE, U], DT, name=f"e_{b}", tag="ep")
                nc.sync.dma_start(
                    out=epair,
                    in_=embT[b : b + nb2].rearrange("b (k p) t -> p b k t", p=128),
                )
                epairs[b] = epair
                e = epair[:, 0]
            else:
                e = epairs[b - 1][:, 1]
            hT[b] = hp.tile([128, JT, U], H_DT, name=f"hT_{b}", tag="hT")
            # the last batch's stores go on the (idle by then) sync queue, in
            # quarters so the final store latency is short
            if b == BP - 1 or (b >= 4 and b % 2 == 0):
                store_eng = nc.sync
            else:
                store_eng = nc.gpsimd
            nstores = 4 if b == BP - 1 else 2
            jper = JT // nstores
            for jt in range(JT):
                ps = fc1ps.tile([128, U], F32, name=f"ps_{b}_{jt}", tag="fc1")
                nc.tensor.matmul(
                    ps, lhsT=w1e_t[:, 0, ts(jt, 128)], rhs=e[:, 0, :],
                    start=True, stop=False,
                )
                nc.tensor.matmul(
                    ps, lhsT=w1e_t[:, 1, ts(jt, 128)], rhs=e[:, 1, :],
                    start=False, stop=True,
                )
                bias_ap = bias_t[:, jt * BP + b : jt * BP + b + 1]
                if jt < JT - NDVE:
                    nc.scalar.activation(hT[b][:, jt, :], ps, AF.Relu, bias=bias_ap)
                else:
                    nc.vector.tensor_scalar(
                        out=hT[b][:, jt, :], in0=ps, scalar1=bias_ap,
                        scalar2=0.0, op0=ALU.add, op1=ALU.max,
                    )
                if (jt + 1) % jper == 0:
                    q = jt // jper
                    store_eng.dma_start(
                        out=hT_out[b, ts(q, jper * 128), :].rearrange(
                            "(k p) t -> p k t", p=128
                        ),
                        in_=hT[b][:, ts(q, jper), :],
                    )

        def stage_fc2(b):
            """FC2 matmuls into [64, U] logits (rows 12..63 zero pad) + exp."""
            psL[b] = lps.tile([32, U], F32, name=f"psL_{b}", tag="psL")
            for jt in range(JT):
                nc.tensor.matmul(
                    psL[b], lhsT=w2p_t[:, ts(jt, 32)], rhs=hT[b][:, jt, :],
                    start=(jt == 0), stop=(jt == JT - 1),
                )
            expS[b] = smx.tile([32, U], DT, name=f"exp_{b}", tag="exp", bufs=4)
            nc.scalar.activation(expS[b], psL[b], AF.Exp, bias=b2s_t[0:32, 0:1])

        def stage_sum(b):
            """per-t softmax denominator, reciprocal, ragged-mask fold."""
            ps_sum = smps.tile([1, U], F32, name=f"pssum_{b}", tag="pssum")
            nc.tensor.matmul(ps_sum, lhsT=sum12_t, rhs=expS[b], start=True, stop=True)
            recip = smx.tile([1, U], F32, name=f"recip_{b}", tag="recip")
            nc.vector.reciprocal_approx_fast(out=recip, in_=ps_sum)
            scl[b] = smx.tile([1, U], DT, name=f"scaled_{b}", tag="scaled")
            nc.vector.tensor_mul(scl[b], recip, mfin_t[:, b * U : (b + 1) * U])

        def stage_fin(b):
            """broadcast scale across the 12 logit rows, final probs, out."""
            ps_b = bps.tile([ACTN, U], F32, name=f"psb_{b}", tag="psb")
            nc.tensor.matmul(ps_b, lhsT=ones12_t, rhs=scl[b], start=True, stop=True)
            probsT = smx.tile([ACTN, U], F32, name=f"probsT_{b}", tag="probsT")
            nc.vector.tensor_mul(probsT, expS[b][0:ACTN, :].bitcast(F32), ps_b)
            (nc.sync if b >= BP - 2 else nc.gpsimd).dma_start(
                out=probsT_out[b], in_=probsT
            )

        # prologue: corevec halves interleaved with FC1(0) so the PE never
        # waits on the second w1c half; the cv PSUM banks free afterwards
        with tc.tile_pool(name="cvps", bufs=1, space="PSUM") as cvps:
            emit_corevec(0, cvps)
            stage_fc1(0)
            emit_corevec(1, cvps)
        lps = ctx.enter_context(tc.tile_pool(name="lps", bufs=2, space="PSUM"))
        smps = ctx.enter_context(tc.tile_pool(name="smps", bufs=1, space="PSUM"))
        bps = ctx.enter_context(tc.tile_pool(name="bps", bufs=1, space="PSUM"))
        smx = ctx.enter_context(tc.tile_pool(name="smx", bufs=2))

        # steady pipeline: FC2(i-2) | sum(i-3) | fin(i-4) | FC1(i)
        stage_fc1(1)
        for i in range(2, BP + 4):
            if 0 <= i - 2 < BP:
                stage_fc2(i - 2)
            if 0 <= i - 3 < BP:
                stage_sum(i - 3)
            if 0 <= i - 4 < BP:
                stage_fin(i - 4)
            if i < BP:
                stage_fc1(i)

    nc.compile()
    return nc


_NC = None


def _get_nc():
    global _NC
    if _NC is None:
        _NC = _build()
    return _NC


def _make_in_maps(core_output, own_unit_embeddings, nr_units, nr_own_flags,
                  W1, b1, W2, b2):
    f32 = np.float32
    emb = np.ascontiguousarray(own_unit_embeddings, dtype=f32)
    core = np.ascontiguousarray(core_output, dtype=f32).reshape(B, CORE)
    W1 = np.ascontiguousarray(W1, dtype=f32)
    b1 = np.ascontiguousarray(b1, dtype=f32)
    W2 = np.ascontiguousarray(W2, dtype=f32)
    b2 = np.ascontiguousarray(b2, dtype=f32)
    units = np.asarray(nr_units).reshape(B).astype(f32)
    flags = np.asarray(nr_own_flags).reshape(B).astype(f32)

    w1e = np.ascontiguousarray(W1[:UNIT])
    w1c = np.ascontiguousarray(W1[UNIT:])
    w2p = np.zeros((128, JT * 32), f32)
    for k in range(JT):
        w2p[:, 32 * k : 32 * k + ACTN] = W2[128 * k : 128 * (k + 1)]
    b1r = np.ascontiguousarray(b1.reshape(JT, 128).T)
    b2s = np.zeros((128, 1), f32)
    b2s[:ACTN, 0] = b2
    sum12 = np.zeros((32, 1), f32)
    sum12[:ACTN, 0] = 1.0
    ones12 = np.ones((1, ACTN), f32)
    id8 = np.eye(8, dtype=f32)
    idx = np.arange(U, dtype=f32)[None, :]
    valid = (idx >= flags[:, None]) & (idx < units[:, None])       # [B, U]
    maskv = np.where(valid, np.float32(1.0), np.float32(1e-9))

    shared = dict(w1e=w1e, w1c=w1c, w2p=w2p, b1r=b1r, b2s=b2s, sum12=sum12,
                  ones12=ones12, id8=id8)

    in_maps = []
    for c in range(NCORES):
        sl = slice(c * BP, (c + 1) * BP)
        embT_c = np.ascontiguousarray(emb[sl].transpose(0, 2, 1))
        coreT_c = np.ascontiguousarray(core[sl].T)
        maskf_c = np.ascontiguousarray(maskv[sl].reshape(1, BP * U))
        in_maps.append(dict(embT=embT_c, coreT=coreT_c, maskf=maskf_c, **shared))
    return in_maps


def _gather(results):
    probs = np.empty((B, U, ACTN), np.float32)
    h = np.empty((B, U, HID), np.float32)
    for c, r in enumerate(results):
        sl = slice(c * BP, (c + 1) * BP)
        h[sl] = r["hT_out"].transpose(0, 2, 1)
        probs[sl] = r["probsT_out"].transpose(0, 2, 1)
    return probs, h


def run(inputs, trace=False, **kw):
    """Build + run on 8 cores. Returns (probs, h), BassKernelResults."""
    nc = _get_nc()
    in_maps = _make_in_maps(**inputs)
    res = run_bass_kernel_spmd(nc, in_maps, list(range(NCORES)), trace=trace, **kw)
    return _gather(res.results), res


def kernel(core_output, own_unit_embeddings, nr_units, nr_own_flags, W1, b1, W2, b2):
    (probs, h), _ = run(dict(
        core_output=core_output, own_unit_embeddings=own_unit_embeddings,
        nr_units=nr_units, nr_own_flags=nr_own_flags, W1=W1, b1=b1, W2=W2, b2=b2,
    ))
    return probs, h
